# revision 39
# baseline (speedup 1.0000x reference)
"""Trainium2 Bass kernel: single-head attention with RoPE and the reference's
multiplicative causal mask (masked logits stay 0 -> exp(0)=1, so masked
positions contribute exp(0)=1 to softmax -- attention is dense over the
upper triangle too, but those probabilities are a constant 1/Z).

Sharding: 8 cores = 4 batches x 2 row-parity halves. Core (b, h) owns the
interleaved rows x[b, h::2] -- with this split the causal-mask tile classes
are identical on every core, so fully-masked S^T tiles are skipped
statically (same SPMD graph everywhere) and their P==1 contribution enters
as a per-dout constant (onesum) plus a denominator offset.

Per core: project K for its 1024 rows (bf16 matmuls, fp32 PSUM), RoPE
on-chip, AllGather roped K within the 2-core pair; V projection streams Wv
just-in-time in output-block-major order (keeps K-phase HBM under the
roofline), gathers V at phase end; Q projection + rope; then S^T = K@Q^T,
P = exp(mask*S^T/sqrt(S)), O^T = V^T@P^T / denom. Output is O^T per core;
the host transposes and reassembles.
"""

import sys

for _p in ("/opt/trn_rl_repo", "/root/.axon_site/_ro/trn_rl_repo"):
    if _p not in sys.path:
        sys.path.append(_p)

import math

import ml_dtypes
import numpy as np

BF16 = ml_dtypes.bfloat16

B, S, D = 4, 2048, 2048
NOWN = 1024  # query rows per core
P = 128  # partitions
KD = D // P  # 16 feature chunks
NCJ = S // P  # 16 key chunks
N_CORES = 8
PAIRS = [[0, 1], [2, 3], [4, 5], [6, 7]]
FB = 512  # matmul moving free-dim block
NB = NOWN // FB  # 2 blocks of own rows
WB = D // FB  # 4 dout blocks for the V projection
SCALE = 1.0 / math.sqrt(S)  # reference scales by sqrt(seq_len), not sqrt(D)

# 128-column mask staircase (identical on every core with interleaved
# rows): for query chunk qc (128 columns) and key chunk jc with
# jcl = jc % 8:  qc < jcl -> fully masked (skipped, P==1);
# qc == jcl -> diagonal (mask applied); qc > jcl -> fully unmasked.
QW = 256  # PV / output quarter width (pair of query chunks)
NQ = NOWN // QW  # 4 quarters
NQC = NOWN // P  # 8 query chunks


def _st_pieces(jcl):
    """S^T compute pieces for a key chunk class: the masked diagonal
    128-chunk plus the fully-unmasked remainder split into <=512 spans."""
    pieces = [(jcl * P, P, True)]
    start = (jcl + 1) * P
    while start < NOWN:
        w = min(FB, NOWN - start)
        pieces.append((start, w, False))
        start += w
    return pieces

_CACHE = {}


def _build():
    import concourse.bass as bass  # noqa: F401
    import concourse.tile as tile
    from concourse import bacc, mybir

    f32 = mybir.dt.float32
    bf16 = mybir.dt.bfloat16

    nc = bacc.Bacc(
        "TRN2", target_bir_lowering=False, debug=False, num_devices=N_CORES
    )

    x_ext = nc.dram_tensor("x_t", [P, KD, NOWN], bf16, kind="ExternalInput").ap()
    wq_ext = nc.dram_tensor("wq", [KD, P, KD, P], bf16, kind="ExternalInput").ap()
    wk_ext = nc.dram_tensor("wk", [KD, P, KD, P], bf16, kind="ExternalInput").ap()
    wv_ext = nc.dram_tensor("wv", [WB, P, KD, FB], bf16, kind="ExternalInput").ap()
    cos_ext = nc.dram_tensor("cos_t", [KD, P, NOWN], bf16, kind="ExternalInput").ap()
    sin_ext = nc.dram_tensor("sin_t", [KD, P, NOWN], bf16, kind="ExternalInput").ap()
    mask_ext = nc.dram_tensor("mask_t", [NCJ, P, P], bf16, kind="ExternalInput").ap()
    # output as bf16 tile blocks (host reassembles + upcasts); halves the
    # write traffic and makes every out-DMA a contiguous 64KB block
    out_ext = nc.dram_tensor("out", [NQ, KD, P, QW], bf16, kind="ExternalOutput").ap()
    import os as _os
    _DBG = _os.environ.get("KERNEL_DEBUG_DUMP") == "1"
    if _DBG:
        dbg_pt = nc.dram_tensor(
            "dbg_pt", [P, NCJ, NOWN], bf16, kind="ExternalOutput"
        ).ap()
        dbg_rc = nc.dram_tensor("dbg_rc", [1, NOWN], f32, kind="ExternalOutput").ap()
        dbg_os = nc.dram_tensor(
            "dbg_os", [P, 7 * KD], f32, kind="ExternalOutput"
        ).ap()

    with tile.TileContext(nc) as tc:
        with (
            tc.tile_pool(name="dram", bufs=1, space="DRAM") as dram,
            tc.tile_pool(name="psum", bufs=5, space="PSUM") as psum,
            tc.tile_pool(name="dnsum", bufs=1, space="PSUM") as dnsum,
            tc.tile_pool(name="persist", bufs=1) as persist,
            tc.tile_pool(name="tmp", bufs=6) as tmp,
            tc.tile_pool(name="csp", bufs=6) as csp,
            tc.tile_pool(name="strm", bufs=6) as strm,
        ):
            kt_local = dram.tile([NCJ // 2, P, KD, P], bf16)
            v_local = dram.tile([NCJ // 2, P, D], bf16)
            # gathered tensors, split in halves so each 2MB gather launches
            # as soon as its half is produced (pipelines with compute)
            kt_ga = dram.tile([2, 4, P, KD, P], bf16)
            kt_gb = dram.tile([2, 4, P, KD, P], bf16)
            v_ga = dram.tile([2, 4, P, D], bf16)
            v_gb = dram.tile([2, 4, P, D], bf16)

            def kt_g(jc):
                h2, jcl = jc // 8, jc % 8
                return (kt_ga if jcl < 4 else kt_gb)[h2, jcl % 4]

            def v_g(jc):
                h2, jcl = jc // 8, jc % 8
                return (v_ga if jcl < 4 else v_gb)[h2, jcl % 4]

            ones_col = persist.tile([P, 1], bf16)
            nc.vector.memset(ones_col, 1.0)
            ones_row = persist.tile([1, P], f32)
            nc.vector.memset(ones_row, 1.0)

            qt_sb = persist.tile([P, KD, NOWN], bf16)

            # x in 8 independent tiles so the chunk DMAs run in parallel
            # (DMAs into one tile serialize on its semaphore chain)
            x_pool = tc.alloc_tile_pool(name="x_pool", bufs=1)
            x_ts = [
                x_pool.tile([P, 2, NOWN], bf16, name=f"x_sb{i}") for i in range(8)
            ]
            x_dma_engines = [
                nc.sync,
                nc.scalar,
                nc.sync,
                nc.scalar,
                nc.gpsimd,
                nc.sync,
                nc.scalar,
                nc.gpsimd,
            ]

            def emit_x_load():
                # two column passes: the K phase's nb=0 units only need the
                # first 512 rows, so the startup gate halves
                for lo, hi in ((0, FB), (FB, NOWN)):
                    for kg in range(8):
                        x_dma_engines[kg].dma_start(
                            out=x_ts[kg][:, :, lo:hi],
                            in_=x_ext[:, kg * 2 : (kg + 1) * 2, lo:hi],
                        )

            def x_ref(k):
                return x_ts[k // 2][:, k % 2, :]

            _panel_engines = [nc.sync, nc.scalar]

            def load_panels(wpool, w_ext, dlow, nb, first=0):
                """first=1: startup unit, k-halved on scalar+gpsimd so the
                first matmul only gates on 256KB. first=2: second unit,
                full tiles on sync+gpsimd. Otherwise baseline alternation."""
                dhigh = dlow + KD // 2
                if first == 1:
                    halves = []
                    for i, d in enumerate((dlow, dlow, dhigh, dhigh)):
                        hk = i % 2
                        e = (nc.scalar, nc.scalar, nc.gpsimd, nc.gpsimd)[i]
                        t = wpool.tile([P, KD // 2, P], bf16, tag="wp")
                        e.dma_start(
                            out=t, in_=w_ext[d, :, hk * 8 : (hk + 1) * 8, :]
                        )
                        halves.append(t)
                    lo_a, lo_b, hi_a, hi_b = halves

                    def w_lo(k):
                        return lo_a[:, k, :] if k < 8 else lo_b[:, k - 8, :]

                    def w_hi(k):
                        return hi_a[:, k, :] if k < 8 else hi_b[:, k - 8, :]

                    return w_lo, w_hi
                if first == 2:
                    e0, e1 = nc.sync, nc.gpsimd
                else:
                    e0 = _panel_engines[dlow % 2]
                    e1 = _panel_engines[(dlow + 1) % 2]
                w_lo_t = wpool.tile([P, KD, P], bf16, tag="wp")
                e0.dma_start(out=w_lo_t, in_=w_ext[dlow])
                w_hi_t = wpool.tile([P, KD, P], bf16, tag="wp")
                e1.dma_start(out=w_hi_t, in_=w_ext[dhigh])
                return (lambda k: w_lo_t[:, k, :]), (lambda k: w_hi_t[:, k, :])

            def cs_load(dlow, sl):
                dhigh = dlow + KD // 2
                tiles = []
                for name, src_ in (
                    ("ct", cos_ext[dlow]),
                    ("st", sin_ext[dlow]),
                    ("ch", cos_ext[dhigh]),
                    ("sh", sin_ext[dhigh]),
                ):
                    t = csp.tile(
                        [P, sl.stop - sl.start], bf16, tag="cs", name=f"cs_{name}"
                    )
                    nc.scalar.dma_start(out=t, in_=src_[:, sl])
                    tiles.append(t)
                return tiles

            def rope_pair(panels, dlow, nb, cs_tiles, out_ap, post):
                """One (dlow, nb) unit: two projection chains + rope."""
                dhigh = dlow + KD // 2
                sl = slice(nb * FB, (nb + 1) * FB)
                cos_t, sin_t, cos_h, sin_h = cs_tiles
                w_lo, w_hi = panels
                ps_lo = psum.tile([P, FB], f32, tag="ps", name=f"plo{dlow}{nb}")
                for k in range(KD):
                    nc.tensor.matmul(
                        ps_lo,
                        lhsT=w_lo(k),
                        rhs=x_ref(k)[:, sl],
                        start=(k == 0),
                        stop=(k == KD - 1),
                    )
                ps_hi = psum.tile([P, FB], f32, tag="ps", name=f"phi{dlow}{nb}")
                for k in range(KD):
                    nc.tensor.matmul(
                        ps_hi,
                        lhsT=w_hi(k),
                        rhs=x_ref(k)[:, sl],
                        start=(k == 0),
                        stop=(k == KD - 1),
                    )
                # rope low half: out = lo*cos_l - hi*sin_l
                t1 = tmp.tile([P, FB], f32, tag="t", name=f"t1{dlow}{nb}")
                nc.vector.tensor_mul(t1, ps_lo, cos_t)
                t2 = tmp.tile([P, FB], f32, tag="t", name=f"t2{dlow}{nb}")
                nc.vector.tensor_mul(t2, ps_hi, sin_t)
                o_lo = out_ap(dlow, nb)
                nc.vector.tensor_sub(o_lo, t1, t2)
                if post is not None:
                    post(dlow, nb, o_lo)
                # rope high half: out = hi*cos_h + lo*sin_h
                t3 = tmp.tile([P, FB], f32, tag="t", name=f"t3{dlow}{nb}")
                nc.vector.tensor_mul(t3, ps_hi, cos_h)
                t4 = tmp.tile([P, FB], f32, tag="t", name=f"t4{dlow}{nb}")
                nc.vector.tensor_mul(t4, ps_lo, sin_h)
                o_hi = out_ap(dhigh, nb)
                nc.vector.tensor_add(o_hi, t3, t4)
                if post is not None:
                    post(dhigh, nb, o_hi)

            # ---- K projection + rope -> kt_local; gather per half ----
            def k_out(dc, nb):
                return strm.tile([P, FB], bf16, tag="ro", name=f"kt_{dc}_{nb}")

            def k_post(dc, nb, t):
                for jj in range(FB // P):
                    nc.gpsimd.dma_start(
                        out=kt_local[nb * 4 + jj][:, dc, :],
                        in_=t[:, jj * P : (jj + 1) * P],
                    )

            def emit_kt_gather(half, out_t):
                nc.gpsimd.collective_compute(
                    "AllGather",
                    mybir.AluOpType.bypass,
                    replica_groups=PAIRS,
                    ins=[kt_local[half * 4 : (half + 1) * 4].opt()],
                    outs=[out_t.opt()],
                )

            # Wv streamed block-major during the V phase; prefetch block 0
            # late in the K phase.
            wv_pool = tc.alloc_tile_pool(name="wv_pool", bufs=4)
            wv_blocks = {}

            def load_wv_block(wb, eng):
                t = wv_pool.tile([P, KD, FB], bf16, tag="wvb", name=f"wvb{wb}")
                eng.dma_start(out=t, in_=wv_ext[wb])
                wv_blocks[wb] = t

            # nb-outer so each half of kt_local completes early and its 2MB
            # gather pipelines under the remaining projections
            with tc.tile_pool(name="wk_pool", bufs=10) as wkp:
                pre = [
                    load_panels(wkp, wk_ext, d, 0, first=d + 1) for d in range(2)
                ]
                emit_x_load()
                unit = 0
                for nb in range(NB):
                    for dlow in range(KD // 2):
                        if nb == 0 and dlow < 2:
                            panels = pre[dlow]
                        else:
                            panels = load_panels(wkp, wk_ext, dlow, nb)
                        cs_tiles = cs_load(dlow, slice(nb * FB, (nb + 1) * FB))
                        rope_pair(panels, dlow, nb, cs_tiles, k_out, k_post)
                        if unit == 14:
                            load_wv_block(0, nc.gpsimd)
                        unit += 1
                    emit_kt_gather(nb, kt_ga if nb == 0 else kt_gb)

            # ---- V projection (block-major, Wv JIT) -> v_local ----
            # two row-half passes so each half's gather launches mid-phase
            # and the gathered V is in SBUF well before the attention block
            for half in range(2):
                for wb in range(WB):
                    for wbn in (wb, wb + 1, wb + 2):
                        if wbn < WB and wbn not in wv_blocks:
                            load_wv_block(wbn, nc.scalar)
                    wvb = wv_blocks[wb]
                    for ncc in range(half * 4, half * 4 + 4):
                        ps_v = psum.tile([P, FB], f32, tag="ps")
                        for k in range(KD):
                            nc.tensor.matmul(
                                ps_v,
                                lhsT=x_ref(k)[:, ncc * P : (ncc + 1) * P],
                                rhs=wvb[:, k, :],
                                start=(k == 0),
                                stop=(k == KD - 1),
                            )
                        v_t = strm.tile([P, FB], bf16, tag="vo")
                        nc.vector.tensor_copy(v_t, ps_v)
                        nc.gpsimd.dma_start(
                            out=v_local[ncc][:, wb * FB : (wb + 1) * FB], in_=v_t
                        )
                nc.gpsimd.collective_compute(
                    "AllGather",
                    mybir.AluOpType.bypass,
                    replica_groups=PAIRS,
                    ins=[v_local[half * 4 : (half + 1) * 4].opt()],
                    outs=[(v_ga if half == 0 else v_gb).opt()],
                )
            wv_pool.release()

            # ---- Q projection + rope (covers the V gathers) ----
            def q_out(dc, nb):
                return qt_sb[:, dc, nb * FB : (nb + 1) * FB]

            with tc.tile_pool(name="wq_pool", bufs=10) as wqp:
                for dlow in range(KD // 2):
                    cs_full = cs_load(dlow, slice(0, NOWN))
                    panels = load_panels(wqp, wq_ext, dlow, 0)
                    for nb in range(NB):
                        sl = slice(nb * FB, (nb + 1) * FB)
                        cs_tiles = [t[:, sl] for t in cs_full]
                        rope_pair(panels, dlow, nb, cs_tiles, q_out, None)
            x_pool.release()

            # ---- Attention ----
            with (
                tc.tile_pool(name="v2_pool", bufs=1) as v2p,
                tc.tile_pool(name="pt_pool", bufs=1) as ptp,
                tc.tile_pool(name="slab", bufs=6) as slab,
                tc.tile_pool(name="mskp", bufs=3) as mskp,
                tc.tile_pool(name="outp", bufs=4) as outp,
                tc.tile_pool(name="smallp", bufs=2) as smallp,
            ):
                v2_sb = v2p.tile([P, NCJ, D], bf16)
                # half-A chunks first (their gather finishes first)
                for jc in (0, 1, 2, 3, 8, 9, 10, 11, 4, 5, 6, 7, 12, 13, 14, 15):
                    nc.gpsimd.dma_start(out=v2_sb[:, jc, :], in_=v_g(jc))

                pt_sb = ptp.tile([P, NCJ, NOWN], bf16)

                def s_piece(jc, st, w, kt_slab, msk):
                    ps_s = psum.tile([P, w], f32, tag="ps", name=f"ps_s{jc}{st}")
                    for k in range(KD):
                        nc.tensor.matmul(
                            ps_s,
                            lhsT=kt_slab[:, k, :],
                            rhs=qt_sb[:, k, st : st + w],
                            start=(k == 0),
                            stop=(k == KD - 1),
                        )
                    if msk is not None:
                        tm = tmp.tile([P, w], f32, tag="t", name=f"tm{jc}{st}")
                        nc.vector.tensor_mul(tm, ps_s, msk)
                        esrc = tm
                    else:
                        esrc = ps_s
                    nc.scalar.activation(
                        out=pt_sb[:, jc, st : st + w],
                        in_=esrc,
                        func=mybir.ActivationFunctionType.Exp,
                        scale=SCALE,
                    )

                for jc in range(NCJ):
                    kt_slab = slab.tile([P, KD, P], bf16, tag="slab")
                    nc.sync.dma_start(out=kt_slab, in_=kt_g(jc))
                    msk = mskp.tile([P, P], bf16, tag="m")
                    nc.scalar.dma_start(out=msk, in_=mask_ext[jc])
                    for st, w, masked in _st_pieces(jc % 8):
                        s_piece(jc, st, w, kt_slab, msk if masked else None)

                # onesum stages (key-chunk classes jcl=1..7), all regions of
                # one PSUM tile, then cumulative sums on DVE:
                # os[qc] = sum of stages jcl > qc
                ps_os = dnsum.tile([P, 7 * KD], f32, tag="os", name="pso")
                for si, jcl in enumerate(range(1, 8)):
                    for dc in range(KD):
                        col = si * KD + dc
                        for idx, jc in enumerate((jcl, jcl + 8)):
                            nc.tensor.matmul(
                                ps_os[:, col : col + 1],
                                lhsT=v2_sb[:, jc, dc * P : (dc + 1) * P],
                                rhs=ones_col,
                                start=(idx == 0),
                                stop=(idx == 1),
                            )
                parts_sb = smallp.tile(
                    [P, 7 * KD], f32, tag="osp", name="osp", bufs=1
                )
                nc.vector.tensor_copy(parts_sb, ps_os)

                def part(jcl):
                    return parts_sb[:, (jcl - 1) * KD : jcl * KD]

                os_of_qc = {6: part(7)}
                for qc in range(5, -1, -1):
                    t = smallp.tile(
                        [P, KD], f32, tag=f"osc{qc}", name=f"osc{qc}", bufs=1
                    )
                    nc.vector.tensor_add(t, os_of_qc[qc + 1], part(qc + 1))
                    os_of_qc[qc] = t

                # denominators per 128-query chunk as regions of one PSUM
                # tile; skipped chunks contribute (7-qc)*256 exact ones
                dn_t = dnsum.tile([1, NOWN], f32, tag="dn", name="psd")
                for qc in range(NQC):
                    jcs = [jc for jc in range(NCJ) if jc % 8 <= qc]
                    for idx, jc in enumerate(jcs):
                        nc.tensor.matmul(
                            dn_t[:, qc * P : (qc + 1) * P],
                            lhsT=ones_col,
                            rhs=pt_sb[:, jc, qc * P : (qc + 1) * P],
                            start=(idx == 0),
                            stop=(idx == len(jcs) - 1),
                        )
                dfix = smallp.tile([1, NOWN], f32, tag="dfix", name="dfix", bufs=1)
                for qc in range(NQC):
                    nc.vector.tensor_scalar_add(
                        dfix[:, qc * P : (qc + 1) * P],
                        dn_t[:, qc * P : (qc + 1) * P],
                        float((7 - qc) * 2 * P),
                    )
                recip_sb = smallp.tile(
                    [1, NOWN], f32, tag="rc", name="recip", bufs=1
                )
                nc.vector.reciprocal(recip_sb, dfix)

                def pv_chain(q, dc):
                    # two 128-wide regions, each accumulated as a CONTIGUOUS
                    # group (interleaving two open accumulation groups within
                    # one PSUM bank corrupts results on hardware)
                    ps_o = psum.tile([P, QW], f32, tag="ps", name=f"pso{q}{dc}")
                    lo = slice(2 * q * P, (2 * q + 1) * P)
                    hi = slice((2 * q + 1) * P, (2 * q + 2) * P)
                    full = [jc for jc in range(NCJ) if jc % 8 <= 2 * q]
                    his = [2 * q + 1, 2 * q + 9] + full
                    for idx, jc in enumerate(his):
                        nc.tensor.matmul(
                            ps_o[:, P:QW],
                            lhsT=v2_sb[:, jc, dc * P : (dc + 1) * P],
                            rhs=pt_sb[:, jc, hi],
                            start=(idx == 0),
                            stop=(idx == len(his) - 1),
                        )
                    for idx, jc in enumerate(full):
                        nc.tensor.matmul(
                            ps_o[:, 0:P],
                            lhsT=v2_sb[:, jc, dc * P : (dc + 1) * P],
                            rhs=pt_sb[:, jc, lo],
                            start=(idx == 0),
                            stop=(idx == len(full) - 1),
                        )
                    return ps_o

                rbs = {}

                def emit_rbs():
                    # reciprocal broadcast via fp32 outer product; emitted a
                    # PV chain late so the DVE reciprocals are done
                    for q in range(NQ):
                        ps_rb = psum.tile([P, QW], f32, tag="ps", name=f"prb{q}")
                        nc.tensor.matmul(
                            ps_rb,
                            lhsT=ones_row,
                            rhs=recip_sb[:, q * QW : (q + 1) * QW],
                            start=True,
                            stop=True,
                        )
                        rb = smallp.tile(
                            [P, QW], f32, tag="rbs", name=f"rb{q}", bufs=NQ
                        )
                        nc.vector.tensor_copy(rb, ps_rb)
                        rbs[q] = rb

                _out_engines = [nc.gpsimd, nc.sync, nc.scalar]

                def emit_scale(q, dc, ps_o):
                    o_st = outp.tile([P, QW], bf16, tag="o", name=f"ost{q}{dc}")
                    qc_lo, qc_hi = 2 * q, 2 * q + 1
                    nc.vector.scalar_tensor_tensor(
                        out=o_st[:, 0:P],
                        in0=ps_o[:, 0:P],
                        scalar=os_of_qc[qc_lo][:, dc : dc + 1],
                        in1=rbs[q][:, 0:P],
                        op0=mybir.AluOpType.add,
                        op1=mybir.AluOpType.mult,
                    )
                    if qc_hi in os_of_qc:
                        nc.vector.scalar_tensor_tensor(
                            out=o_st[:, P:QW],
                            in0=ps_o[:, P:QW],
                            scalar=os_of_qc[qc_hi][:, dc : dc + 1],
                            in1=rbs[q][:, P:QW],
                            op0=mybir.AluOpType.add,
                            op1=mybir.AluOpType.mult,
                        )
                    else:
                        nc.vector.tensor_mul(
                            o_st[:, P:QW], ps_o[:, P:QW], rbs[q][:, P:QW]
                        )
                    _out_engines[dc % 3].dma_start(
                        out=out_ext[q, dc], in_=o_st
                    )

                pend = pv_chain(0, 0)
                emit_rbs()
                emit_scale(0, 0, pend)
                for dc in range(1, KD):
                    emit_scale(0, dc, pv_chain(0, dc))
                for q in (3, 2, 1):
                    for dc in range(KD):
                        emit_scale(q, dc, pv_chain(q, dc))

                if _DBG:
                    nc.sync.dma_start(out=dbg_pt, in_=pt_sb)
                    nc.sync.dma_start(out=dbg_rc, in_=recip_sb)
                    nc.sync.dma_start(out=dbg_os, in_=parts_sb)

    nc.compile()
    return nc


def _prep_inputs(x, cos, sin, Wq, Wk, Wv):
    """Host-side sharding/layout prep. Returns in_maps for 8 cores."""
    x = np.asarray(x, dtype=np.float32)
    cos = np.asarray(cos, dtype=np.float32)
    sin = np.asarray(sin, dtype=np.float32)

    def w_panels(w):
        # W.T [din, dout] -> [dc, p_din, k_din, c_dout] with d = k*128+p
        wt = np.ascontiguousarray(np.asarray(w, dtype=np.float32).T).astype(BF16)
        return np.ascontiguousarray(
            wt.reshape(KD, P, KD, P).transpose(2, 1, 0, 3)
        )

    wq_p = w_panels(Wq)
    wk_p = w_panels(Wk)
    # Wv.T [din, dout] -> [wb, p, k, c_dout]  (dout-block-major)
    wv_flat = np.ascontiguousarray(
        np.asarray(Wv, dtype=np.float32)
        .T.astype(BF16)
        .reshape(KD, P, D)
        .transpose(1, 0, 2)
    )
    wv_p = np.ascontiguousarray(
        wv_flat.reshape(P, KD, WB, FB).transpose(2, 0, 1, 3)
    )

    # global row index of gathered slot s: pair rank h2 = s // NOWN owns the
    # rows with parity h2, so j_global(s) = 2*(s % NOWN) + h2
    slot = np.arange(S, dtype=np.int64)
    j_global = 2 * (slot % NOWN) + slot // NOWN

    in_maps = []
    for c in range(N_CORES):
        b, h = divmod(c, 2)
        rows = slice(h, None, 2)  # interleaved rows: h, h+2, h+4, ...
        xt = np.ascontiguousarray(
            x[b, rows, :].T.astype(BF16).reshape(KD, P, NOWN).transpose(1, 0, 2)
        )
        cos_t = np.ascontiguousarray(cos[rows].T.astype(BF16).reshape(KD, P, NOWN))
        sin_t = np.ascontiguousarray(sin[rows].T.astype(BF16).reshape(KD, P, NOWN))
        i_global = 2 * np.arange(NOWN, dtype=np.int64) + h
        # per jc, only the diagonal 128-query chunk (qc == jc%8) needs mask
        mask_t = np.empty((NCJ, P, P), dtype=BF16)
        for jc in range(NCJ):
            qc = jc % 8
            jg = j_global[jc * P : (jc + 1) * P][:, None]
            ig = i_global[qc * P : (qc + 1) * P][None, :]
            mask_t[jc] = (jg <= ig).astype(BF16)
        in_maps.append(
            {
                "x_t": xt,
                "wq": wq_p,
                "wk": wk_p,
                "wv": wv_p,
                "cos_t": cos_t,
                "sin_t": sin_t,
                "mask_t": mask_t,
            }
        )
    return in_maps


def _run(in_maps, trace=False, tmpdir=None):
    from concourse.bass_utils import run_bass_kernel_spmd

    if "nc" not in _CACHE:
        _CACHE["nc"] = _build()
    nc = _CACHE["nc"]
    return run_bass_kernel_spmd(
        nc, in_maps, list(range(N_CORES)), trace=trace, tmpdir=tmpdir
    )


def _assemble(blocks):
    # [NQ, KD, P, QW] bf16 tile blocks -> [NOWN, D] fp32
    return (
        np.asarray(blocks)
        .transpose(0, 3, 1, 2)
        .reshape(NOWN, D)
        .astype(np.float32)
    )


def kernel(x, cos, sin, Wq, Wk, Wv):
    in_maps = _prep_inputs(x, cos, sin, Wq, Wk, Wv)
    res = _run(in_maps, trace=False)
    out = np.empty((B, S, D), dtype=np.float32)
    for c in range(N_CORES):
        b, h = divmod(c, 2)
        out[b, h::2, :] = _assemble(res.results[c]["out"])
    return out
